# revision 64
# baseline (speedup 1.0000x reference)
"""Trainium2 Bass kernel for nn_EDTransformer (encoder-decoder transformer).

Sharding: 8 cores = 4 batch items x 2 sequence halves.
 - Each core owns (item b, half h): computes Q/scores/AV/Wo/MLP/LN for its
   256 local positions; K/V computed for the LOCAL half only and completed
   via a 2-core AllGather of K/V per attention block.
 - Decoder self+cross attention share one weight load per layer.
 - Unembedding sharded over vocab (4000 rows/core, 8 chunks of 500),
   computed TRANSPOSED (positions on partitions) so the softmax denominator
   comes from the Act engine accumulator and the normalize is a
   per-partition scale; denominator summed via one 8-core AllReduce.
 - Weights pre-tiled host-side so every DMA reads contiguous >=2KB runs
   per partition; weight loads spread across sync/scalar queues.
Dtypes: fp16 matmul operands, fp32 PSUM, fp32 residual + LN stats,
 fp16 output (cast to fp32 on host).
"""
import os
import sys

sys.path.insert(0, '/opt/trn_rl_repo')
import numpy as np

import concourse.bacc as bacc
import concourse.tile as tile
import concourse.mybir as mybir
from concourse.bass_utils import run_bass_kernel_spmd

DT = mybir.dt
F16 = DT.float16
F32 = DT.float32
AF = mybir.ActivationFunctionType

N_CORES = 8
P = 128
DE = 1024           # model dim (8 ptiles)
KO = DE // P        # 8
DMLP = 4096         # mlp dim
MO = DMLP // P      # 32
H = 16              # heads
DA = 64             # attn dim per head
L = 512             # sequence length
LL = 256            # local positions per core
KT = L // P         # 4 key tiles
NV = 32000
NVC = NV // N_CORES  # 4000 vocab rows per core
VC = 500            # vocab chunk (8 chunks of 500)
NVCH = NVC // VC    # 8
LENC = 2
LDEC = 2
EPS = 1e-5

PAIR_GROUPS = [[0, 1], [2, 3], [4, 5], [6, 7]]
ALL_GROUP = [list(range(N_CORES))]

_CACHE = {}


# ----------------------------------------------------------------------------
# device program
# ----------------------------------------------------------------------------

def _kv_proj_ag(nc, pools, W, kvin16, bufs, kfull, vt16, wb=True):
    """Project K/V from local stream and pair-AllGather (k first, v second).

    kvin16: [128, KO, LL] local stream.
    kfull : [128, KO, L]  (partitions = 2h x 64a rows)
    vt16  : [128, KT, H*DA] (partitions = key positions)
    bufs  : (agin_k, agout_k, agin_v, agout_v) dram tensors
    """
    agin, agout = bufs
    p256 = pools['p256']
    p512 = pools['p512']
    kloc = pools['att'].tile([P, KO, LL], F16, tag='kloc')
    for p2 in range(KO // 2):
        ps = p256.tile([P, 2, LL], F32, tag='p256')
        for j in range(2):
            pr = 2 * p2 + j
            for k in range(KO):
                nc.tensor.matmul(ps[:, j, :],
                                 W['wk'][:, k, pr * P:(pr + 1) * P],
                                 kvin16[:, k, :], start=(k == 0),
                                 stop=(k == KO - 1))
        nc.vector.tensor_copy(kloc[:, 2 * p2:2 * p2 + 2, :], ps[:])
    nc.gpsimd.dma_start(
        agin[:, 0:2048].rearrange('p (a b) -> p a b', a=KO), kloc[:])
    vloc = pools['att'].tile([P, 2, H * DA], F16, tag='vloc')
    for lc in range(2):
        for nch in range(2):
            ps = p512.tile([P, 512], F32, tag='p512', bufs=2)
            for k in range(KO):
                nc.tensor.matmul(ps[:], kvin16[:, k, lc * P:(lc + 1) * P],
                                 W['wv'][:, k, nch * 512:(nch + 1) * 512],
                                 start=(k == 0), stop=(k == KO - 1))
            nc.vector.tensor_copy(vloc[:, lc, nch * 512:(nch + 1) * 512], ps[:])
    nc.gpsimd.dma_start(
        agin[:, 2048:4096].rearrange('p (a b) -> p a b', a=2), vloc[:])
    nc.gpsimd.collective_compute(
        "AllGather", mybir.AluOpType.bypass,
        ins=[agin[:]], outs=[agout[:]], replica_groups=PAIR_GROUPS)
    if wb:
        _kv_writeback(nc, bufs, kfull, vt16)


def _kv_writeback(nc, bufs, kfull, vt16):
    _, agout = bufs
    for r in range(2):
        nc.gpsimd.dma_start(
            kfull[:, :, r * LL:(r + 1) * LL],
            agout[r, :, 0:2048].rearrange('p (a b) -> p a b', a=KO))
        nc.gpsimd.dma_start(
            vt16[:, r * 2:(r + 1) * 2, :],
            agout[r, :, 2048:4096].rearrange('p (a b) -> p a b', a=2))


def _attn_core(nc, pools, Eres32, qin16, W, kfull, vt16, mask, name):
    """Q projection, scores/softmax/AV per head-pair, Wo accumulate."""
    p256 = pools['p256']
    p512 = pools['p512']
    ones = pools['ones']
    sb = pools['att']

    q16 = sb.tile([P, KO, LL], F16, tag='q16')
    for p2 in range(KO // 2):
        ps = p256.tile([P, 2, LL], F32, tag='p256')
        for j in range(2):
            pr = 2 * p2 + j
            for k in range(KO):
                nc.tensor.matmul(ps[:, j, :],
                                 W['wq'][:, k, pr * P:(pr + 1) * P],
                                 qin16[:, k, :], start=(k == 0),
                                 stop=(k == KO - 1))
        nc.vector.tensor_copy(q16[:, 2 * p2:2 * p2 + 2, :], ps[:])

    y16 = sb.tile([P, KO, LL], F16, tag='y16')
    for pr in range(KO):
        hA, hB = 2 * pr, 2 * pr + 1
        # scores -> exp: one [128, 4, 256] psum + one exp per head
        exp16 = sb.tile([P, 2, KT, LL], F16, tag='exp16', bufs=2)
        for hh in range(2):
            h = hA + hh
            hp = (h % 2) * DA
            for kp in range(2):
                ps = p512.tile([P, 2, LL], F32, tag='psc', bufs=2)
                for ki in range(2):
                    kt = 2 * kp + ki
                    nc.tensor.matmul(
                        ps[:, ki, :],
                        kfull[hp:hp + DA, pr, kt * P:(kt + 1) * P],
                        q16[hp:hp + DA, pr, :], start=True, stop=True)
                nc.scalar.activation(exp16[:, hh, 2 * kp:2 * kp + 2, :],
                                     ps[:], AF.Exp)
        if mask is not None:
            nc.vector.tensor_tensor(
                exp16[:], exp16[:],
                mask[:, None, :, :].to_broadcast((P, 2, KT, LL)),
                mybir.AluOpType.mult)
        # denominators (replicated over partitions via ones matmul)
        pd = p512.tile([P, 2, LL], F32, tag='pd', bufs=2)
        for kt in range(KT):
            nc.tensor.matmul(pd[:], ones[:, :], exp16[:, :, kt, :],
                             start=(kt == 0), stop=(kt == KT - 1))
        ysc = pools['stat'].tile([P, 2, LL], F32, tag='ysc', bufs=2)
        nc.vector.reciprocal_approx_fast(ysc[:], pd[:])
        # AV (2-head column packing)
        ps = p256.tile([P, LL], F32, tag='p256')
        for kt in range(KT):
            nc.tensor.matmul(ps[:DA, :], vt16[:, kt, hA * DA:(hA + 1) * DA],
                             exp16[:, 0, kt, :], start=(kt == 0),
                             stop=(kt == KT - 1), tile_position=(0, 0))
            nc.tensor.matmul(ps[DA:, :], vt16[:, kt, hB * DA:(hB + 1) * DA],
                             exp16[:, 1, kt, :], start=(kt == 0),
                             stop=(kt == KT - 1), tile_position=(0, DA))
        nc.vector.tensor_tensor(y16[:DA, pr, :], ps[:DA, :], ysc[:DA, 0, :],
                                mybir.AluOpType.mult)
        nc.vector.tensor_tensor(y16[DA:, pr, :], ps[DA:, :], ysc[DA:, 1, :],
                                mybir.AluOpType.mult)

    # Wo -> residual: Eres = psum + stream base (the attn input)
    for p2 in range(KO // 2):
        ps = p256.tile([P, 2, LL], F32, tag='p256')
        for j in range(2):
            dt = 2 * p2 + j
            for k in range(KO):
                nc.tensor.matmul(ps[:, j, :],
                                 W['wo'][:, k, dt * P:(dt + 1) * P],
                                 y16[:, k, :], start=(k == 0),
                                 stop=(k == KO - 1))
        nc.vector.tensor_tensor(Eres32[:, 2 * p2:2 * p2 + 2, :], ps[:],
                                qin16[:, 2 * p2:2 * p2 + 2, :],
                                mybir.AluOpType.add)
    tp = pools.get('tapfn')
    if tp:
        tp(f'{name}_q', q16)
        tp(f'{name}_y', y16)


def _load_attn_w(nc, pools, wq_d, wk_d, wv_d, wo_d):
    wq = pools['wqp'].tile([P, KO, DE], F16, tag='wq')
    nc.sync.dma_start(wq[:], wq_d[:])
    wk = pools['wkp'].tile([P, KO, DE], F16, tag='wk')
    nc.scalar.dma_start(wk[:], wk_d[:])
    wv = pools['wvp'].tile([P, KO, DE], F16, tag='wv')
    nc.sync.dma_start(wv[:], wv_d[:])
    wo = pools['wop'].tile([P, KO, DE], F16, tag='wo')
    nc.scalar.dma_start(wo[:], wo_d[:])
    return {'wq': wq, 'wk': wk, 'wv': wv, 'wo': wo}


def _mlp(nc, pools, Eres32, ein16, w1_d, w2_d, name):
    p256 = pools['p256']
    h16 = pools['mlp'].tile([P, MO, LL], F16, tag='h16')
    for c in range(16):
        w1t = pools['w1p'].tile([P, KO, LL], F16, tag='w1t')
        (nc.sync if c % 2 == 0 else nc.scalar).dma_start(w1t[:], w1_d[c])
        ps = p256.tile([P, 2, LL], F32, tag='p256')
        for m in range(2):
            for k in range(KO):
                nc.tensor.matmul(ps[:, m, :], w1t[:, k, m * P:(m + 1) * P],
                                 ein16[:, k, :], start=(k == 0),
                                 stop=(k == KO - 1))
        nc.scalar.activation(h16[:, 2 * c:2 * c + 2, :], ps[:], AF.Relu)
    for dt in range(KO):
        ps = p256.tile([P, LL], F32, tag='p256')
        for half in range(2):
            w2t = pools['w2p'].tile([P, 16, P], F16, tag='w2t')
            (nc.sync if half == 0 else nc.scalar).dma_start(
                w2t[:], w2_d[2 * dt + half])
            for k in range(16):
                nc.tensor.matmul(ps[:], w2t[:, k, :],
                                 h16[:, half * 16 + k, :],
                                 start=(half == 0 and k == 0),
                                 stop=(half == 1 and k == 15))
        nc.vector.tensor_tensor(Eres32[:, dt, :], ps[:], ein16[:, dt, :],
                                mybir.AluOpType.add)


def _ln(nc, pools, Eres32, e16out, name):
    """In-place layernorm over features; writes fp16 copy to e16out."""
    p256 = pools['p256']
    ones = pools['ones']
    stat = pools['stat']

    e16pre = pools['lnp'].tile([P, KO, LL], F16, tag='e16pre')
    nc.vector.tensor_copy(e16pre[:], Eres32[:])
    sq16 = pools['lnp'].tile([P, KO, LL], F16, tag='sq16')
    nc.scalar.square(sq16[:], e16pre[:])
    pss = p256.tile([P, 2, LL], F32, tag='p256')
    for k in range(KO):
        nc.tensor.matmul(pss[:, 0, :], ones[:, :], e16pre[:, k, :],
                         start=(k == 0), stop=(k == KO - 1))
    for k in range(KO):
        nc.tensor.matmul(pss[:, 1, :], ones[:, :], sq16[:, k, :],
                         start=(k == 0), stop=(k == KO - 1))
    psq = pss[:, 1, :]
    mean = stat.tile([P, LL], F32, tag='mean')
    nc.vector.tensor_scalar_mul(mean[:], pss[:, 0, :], 1.0 / DE)
    varn = stat.tile([P, LL], F32, tag='varn')
    nc.vector.tensor_tensor(varn[:], pss[:, 0, :], mean[:],
                            mybir.AluOpType.mult)
    nc.vector.tensor_tensor(varn[:], psq, varn[:], mybir.AluOpType.subtract)
    std = stat.tile([P, LL], F32, tag='std')
    nc.scalar.activation(std[:], varn[:], AF.Sqrt,
                         bias=pools['eps128'], scale=1.0 / (DE - 1))
    inv = stat.tile([P, LL], F32, tag='inv')
    nc.vector.reciprocal_approx_fast(inv[:], std[:])
    ms = stat.tile([P, LL], F32, tag='ms')
    nc.vector.tensor_tensor(ms[:], mean[:], inv[:], mybir.AluOpType.mult)
    t16 = pools['lnp'].tile([P, KO, LL], F16, tag='sq16')
    nc.vector.tensor_tensor(
        t16[:], Eres32[:],
        inv[:, None, :].to_broadcast((P, KO, LL)), mybir.AluOpType.mult)
    nc.vector.tensor_tensor(
        e16out[:], t16[:],
        ms[:, None, :].to_broadcast((P, KO, LL)), mybir.AluOpType.subtract)
    tp = pools.get('tapfn')
    if tp:
        tp(f'{name}_out', e16out)


def build_program(taps=()):
    taps = set(taps)
    nc = bacc.Bacc("TRN2", target_bir_lowering=False, debug=False,
                   num_devices=N_CORES)

    # ---- dram inputs ----
    din = {}

    def dram_in(nm, shape, dt=F16):
        din[nm] = nc.dram_tensor(nm, list(shape), dt, kind="ExternalInput")
        return din[nm]

    z0l16 = dram_in('z0_loc16', [P, KO, LL])
    x0l16 = dram_in('x0_loc16', [P, KO, LL])
    mask_self = dram_in('mask_self', [P, KT, LL])
    for pfx, nl in (('enc', LENC), ('dec', LDEC)):
        for w in ('wqT', 'wkT', 'wvT', 'woT'):
            dram_in(f'{pfx}_{w}', [nl, P, KO, DE])
        dram_in(f'{pfx}_w1T', [nl, 16, P, KO, LL])
        dram_in(f'{pfx}_w2T', [nl, 16, P, 16, P])
    wuT = dram_in('wuT', [64, P, KO, VC])

    # output per core: all 32000 vocab x its 256 local positions
    # [grp, ll, lt, vv*8] fp16 (one 2MB store per grp, 16KB rows)
    outp = nc.dram_tensor('outp', [8, P, 2, 8 * VC], F16,
                          kind="ExternalOutput")

    # internal dram for collectives (reused across attns; gpsimd-serialized)
    agbufs = (nc.dram_tensor('agin', [P, 4096], F16),
              nc.dram_tensor('agout', [2, P, 4096], F16))
    agbufs_d0 = (nc.dram_tensor('agin0', [P, 4096], F16),
                 nc.dram_tensor('agout0', [2, P, 4096], F16))
    # tiny warm-up buffers: trigger CC channel init at t=0
    wu_in = nc.dram_tensor('wu_in', [P, 1], F16)
    wu_p_out = nc.dram_tensor('wu_p_out', [2, P, 1], F16)

    import contextlib
    with tile.TileContext(nc) as tc, contextlib.ExitStack() as octx:
        const = octx.enter_context(tc.tile_pool(name='const', bufs=1))
        ones = const.tile([P, P], F16)
        nc.vector.memset(ones[:], 1.0)
        eps128 = const.tile([P, 1], F32)
        nc.vector.memset(eps128[:], EPS)
        msk = const.tile([P, KT, LL], F16)
        nc.sync.dma_start(msk[:], mask_self[:])

        # warm up the pair CC channels immediately (the only group shape used)
        wtile = const.tile([P, 1], F16)
        nc.vector.memset(wtile[:], 0.0)
        nc.gpsimd.dma_start(wu_in[:], wtile[:])
        nc.gpsimd.collective_compute(
            "AllGather", mybir.AluOpType.bypass,
            ins=[wu_in[:]], outs=[wu_p_out[:]], replica_groups=PAIR_GROUPS)
        xf16 = const.tile([P, KO, LL], F16)

        # ================= layer phase =================
        with contextlib.ExitStack() as ctx:
            stream = ctx.enter_context(tc.tile_pool(name='stream', bufs=1))
            att = ctx.enter_context(tc.tile_pool(name='att', bufs=1))
            mlpp = ctx.enter_context(tc.tile_pool(name='mlpp', bufs=1))
            lnp = ctx.enter_context(tc.tile_pool(name='lnp', bufs=1))
            stat = ctx.enter_context(tc.tile_pool(name='stat', bufs=1))
            wqp = ctx.enter_context(tc.tile_pool(name='wqp', bufs=1))
            wkp = ctx.enter_context(tc.tile_pool(name='wkp', bufs=1))
            wvp = ctx.enter_context(tc.tile_pool(name='wvp', bufs=1))
            wop = ctx.enter_context(tc.tile_pool(name='wop', bufs=1))
            w1p = ctx.enter_context(tc.tile_pool(name='w1p', bufs=2))
            w2p = ctx.enter_context(tc.tile_pool(name='w2p', bufs=2))
            p256 = ctx.enter_context(tc.tile_pool(name='p256', bufs=2,
                                                  space='PSUM'))
            p512 = ctx.enter_context(tc.tile_pool(name='p512', bufs=4,
                                                  space='PSUM'))

            pools = dict(att=att, mlp=mlpp, lnp=lnp, p256=p256, p512=p512,
                         stat=stat, ones=ones, eps128=eps128[:],
                         wqp=wqp, wkp=wkp, wvp=wvp, wop=wop, w1p=w1p, w2p=w2p)

            def tapfn(nm, t):
                if nm not in taps:
                    return
                d = nc.dram_tensor('tap_' + nm, list(t.shape),
                                   t.dtype, kind="ExternalOutput")
                nc.sync.dma_start(d[:], t[:])
            pools['tapfn'] = tapfn

            # ==== dec-l0 self K/V prefetch: fills the CC-init window ====
            eloc_d0 = stream.tile([P, KO, LL], F16, tag='loc_d0')
            nc.sync.dma_start(eloc_d0[:], x0l16[:])
            wk0 = wkp.tile([P, KO, DE], F16, tag='wk')
            nc.scalar.dma_start(wk0[:], din['dec_wkT'][0])
            wv0 = wvp.tile([P, KO, DE], F16, tag='wv')
            nc.sync.dma_start(wv0[:], din['dec_wvT'][0])
            _kv_proj_ag(nc, pools, {'wk': wk0, 'wv': wv0}, eloc_d0,
                        agbufs_d0, None, None, wb=False)

            # ======== encoder ========
            Eres = stream.tile([P, KO, LL], F32, tag='res')
            eloc = stream.tile([P, KO, LL], F16, tag='loc_a')
            nc.sync.dma_start(eloc[:], z0l16[:])

            for l in range(LENC):
                W = _load_attn_w(nc, pools, din['enc_wqT'][l],
                                 din['enc_wkT'][l], din['enc_wvT'][l],
                                 din['enc_woT'][l])
                kfull = att.tile([P, KO, L], F16, tag='kfull')
                vt16 = att.tile([P, KT, H * DA], F16, tag='vt16')
                _kv_proj_ag(nc, pools, W, eloc, agbufs, kfull, vt16)
                _attn_core(nc, pools, Eres, eloc, W, kfull, vt16, None,
                           f'e{l}a')
                eloc = stream.tile([P, KO, LL], F16, tag='loc_b')
                _ln(nc, pools, Eres, eloc, f'e{l}ln1')
                _mlp(nc, pools, Eres, eloc, din['enc_w1T'][l],
                     din['enc_w2T'][l], f'e{l}m')
                eloc = stream.tile([P, KO, LL], F16, tag='loc_a')
                _ln(nc, pools, Eres, eloc, f'e{l}ln2')

            Zloc = stream.tile([P, KO, LL], F16, tag='zloc')
            nc.vector.tensor_copy(Zloc[:], eloc[:])

            # ======== decoder ========
            eloc = eloc_d0

            for l in range(LDEC):
                W = _load_attn_w(nc, pools, din['dec_wqT'][l],
                                 din['dec_wkT'][l], din['dec_wvT'][l],
                                 din['dec_woT'][l])
                # self K/V + AG (layer 0: prefetched at program start)
                kfull_s = att.tile([P, KO, L], F16, tag='kfull')
                vt16_s = att.tile([P, KT, H * DA], F16, tag='vt16')
                if l == 0:
                    _kv_writeback(nc, agbufs_d0, kfull_s, vt16_s)
                else:
                    _kv_proj_ag(nc, pools, W, eloc, agbufs, kfull_s, vt16_s)
                # cross K/V + AG (overlaps self AG; weights shared)
                kfull_c = att.tile([P, KO, L], F16, tag='kfull_c')
                vt16_c = att.tile([P, KT, H * DA], F16, tag='vt16_c')
                _kv_proj_ag(nc, pools, W, Zloc, agbufs, kfull_c, vt16_c)
                # self attention (causal)
                _attn_core(nc, pools, Eres, eloc, W, kfull_s, vt16_s, msk,
                           f'd{l}s')
                eloc = stream.tile([P, KO, LL], F16, tag='loc_b')
                _ln(nc, pools, Eres, eloc, f'd{l}ln1')
                # cross attention
                _attn_core(nc, pools, Eres, eloc, W, kfull_c, vt16_c, None,
                           f'd{l}c')
                eloc = stream.tile([P, KO, LL], F16, tag='loc_b')
                _ln(nc, pools, Eres, eloc, f'd{l}ln2')
                _mlp(nc, pools, Eres, eloc, din['dec_w1T'][l],
                     din['dec_w2T'][l], f'd{l}m')
                eloc = stream.tile([P, KO, LL], F16, tag='loc_a')
                _ln(nc, pools, Eres, eloc, f'd{l}ln3')

            # stash the final stream for the unembed phase
            nc.vector.tensor_copy(xf16[:], eloc[:])

        # ======== unembed phase (position-local: full vocab per core) =====
        with contextlib.ExitStack() as ctx:
            usb = ctx.enter_context(tc.tile_pool(name='usb', bufs=1))
            wup = ctx.enter_context(tc.tile_pool(name='wup', bufs=6))
            u512 = ctx.enter_context(tc.tile_pool(name='u512', bufs=6,
                                                  space='PSUM'))

            expu = usb.tile([P, 2, 64 * VC], F16, tag='expu')
            dacc = usb.tile([P, 2, 64], F32, tag='dacc')
            for vc in range(64):
                wut = wup.tile([P, KO, VC], F16, tag='wut')
                (nc.sync if vc % 2 == 0 else nc.scalar).dma_start(
                    wut[:], wuT[vc])
                for lt in range(2):
                    ps = u512.tile([P, VC], F32, tag='u512')
                    for k in range(KO):
                        nc.tensor.matmul(
                            ps[:], xf16[:, k, lt * P:(lt + 1) * P],
                            wut[:, k, :], start=(k == 0), stop=(k == KO - 1))
                    nc.scalar.activation(
                        expu[:, lt, vc * VC:(vc + 1) * VC], ps[:], AF.Exp,
                        accum_out=dacc[:, lt, vc:vc + 1])
            # denominator: tree-reduce the 64 per-chunk sums (local only)
            cur = dacc
            width = 64
            while width > 1:
                width //= 2
                nxt = usb.tile([P, 2, width], F32, tag=f'dt{width}')
                nc.vector.tensor_tensor(nxt[:], cur[:, :, 0:width],
                                        cur[:, :, width:2 * width],
                                        mybir.AluOpType.add)
                cur = nxt
            binv = usb.tile([P, 2, 1], F32, tag='binv')
            nc.vector.reciprocal_approx_fast(binv[:], cur[:])
            if 'deno' in taps:
                d = nc.dram_tensor('tap_deno', [P, 2, 1], F32,
                                   kind="ExternalOutput")
                nc.sync.dma_start(d[:], cur[:])

            # normalize in place (3 engines), then one big store per group
            dmae = [nc.sync, nc.gpsimd, nc.scalar]
            eng = 0
            for grp in range(8):
                for lt in range(2):
                    sl = expu[:, lt, grp * 8 * VC:(grp + 1) * 8 * VC]
                    i = eng % 3
                    eng += 1
                    if i == 0:
                        nc.vector.tensor_tensor(
                            sl, sl,
                            binv[:, lt, :].to_broadcast((P, 8 * VC)),
                            mybir.AluOpType.mult)
                    elif i == 1:
                        nc.scalar.activation(sl, sl, AF.Copy,
                                             scale=binv[:, lt, :])
                    else:
                        nc.gpsimd.tensor_tensor(
                            sl, sl,
                            binv[:, lt, :].to_broadcast((P, 8 * VC)),
                            mybir.AluOpType.mult)
                dmae[grp % 3].dma_start(
                    outp[grp], expu[:, :, grp * 8 * VC:(grp + 1) * 8 * VC])

    nc.compile()
    return nc


# ----------------------------------------------------------------------------
# host-side prep
# ----------------------------------------------------------------------------

def _to_kimaj(a):
    """[K, M] -> [128, K//128, M] with K = ko*128 + ki."""
    K, M = a.shape
    return np.ascontiguousarray(
        a.reshape(K // P, P, M).transpose(1, 0, 2))


def prep_inputs(inputs):
    f = lambda k: np.asarray(inputs[k], dtype=np.float32)
    We, Wp, Wu = f('We'), f('Wp'), f('Wu')
    x = np.asarray(inputs['x']).astype(np.int64)
    z = np.asarray(inputs['z']).astype(np.int64)

    shared = {}
    for pfx, nl in (('enc', LENC), ('dec', LDEC)):
        Wq, Wk, Wv = f(pfx + '_Wq'), f(pfx + '_Wk'), f(pfx + '_Wv')
        Wo, W1, W2 = f(pfx + '_Wo'), f(pfx + '_W1'), f(pfx + '_W2')
        wq, wk, wv, wo, w1, w2 = [], [], [], [], [], []
        for l in range(nl):
            qa = Wq[l].transpose(2, 0, 1).reshape(DE, H * DA) * (DA ** -0.5)
            ka = Wk[l].transpose(2, 0, 1).reshape(DE, H * DA)
            va = Wv[l].transpose(2, 0, 1).reshape(DE, H * DA)
            wq.append(_to_kimaj(qa))
            wk.append(_to_kimaj(ka))
            wv.append(_to_kimaj(va))
            wo.append(_to_kimaj(Wo[l].T))
            w1k = _to_kimaj(W1[l].T)          # [128, 8, 4096]
            w1.append(np.ascontiguousarray(
                w1k.reshape(P, KO, 16, LL).transpose(2, 0, 1, 3)))
            w2k = _to_kimaj(W2[l].T)          # [128, 32, 1024]
            w2.append(np.ascontiguousarray(
                w2k.reshape(P, 2, 16, 8, P).transpose(3, 1, 0, 2, 4)
                .reshape(16, P, 16, P)))
        shared[f'{pfx}_wqT'] = np.stack(wq).astype(np.float16)
        shared[f'{pfx}_wkT'] = np.stack(wk).astype(np.float16)
        shared[f'{pfx}_wvT'] = np.stack(wv).astype(np.float16)
        shared[f'{pfx}_woT'] = np.stack(wo).astype(np.float16)
        shared[f'{pfx}_w1T'] = np.stack(w1).astype(np.float16)
        shared[f'{pfx}_w2T'] = np.stack(w2).astype(np.float16)

    # full unembedding matrix, chunked: [64][128][8][500]
    wuk = _to_kimaj(Wu.T)                                  # [128, 8, 32000]
    shared['wuT'] = np.ascontiguousarray(
        wuk.reshape(P, KO, 64, VC).transpose(2, 0, 1, 3)).astype(np.float16)

    pos = Wp[:L]  # [512, 1024]
    in_maps = []
    for c in range(N_CORES):
        b, h = c // 2, c % 2
        m = dict(shared)
        for nm, tok in (('z0', z[b]), ('x0', x[b])):
            E0 = (We[tok] + pos).T.astype(np.float32)      # [1024, 512]
            E0k = E0.reshape(KO, P, L)                     # [ko, ki, p]
            loc = E0k[:, :, h * LL:(h + 1) * LL].transpose(1, 0, 2)
            m[nm + '_loc16'] = np.ascontiguousarray(loc).astype(np.float16)
        kglob = np.arange(L)[:, None]
        qglob = (h * LL + np.arange(LL))[None, :]
        msk = (kglob <= qglob).astype(np.float16)          # [512, 256]
        m['mask_self'] = np.ascontiguousarray(
            msk.reshape(KT, P, LL).transpose(1, 0, 2))
        in_maps.append(m)
    return in_maps


def assemble(results):
    """results: per-core dicts with 'outp' [8, 128, 2, 4000] fp16
    (core c covers batch c//2, positions [(c%2)*256, (c%2)*256+256))."""
    out = np.empty((4, NV, L), dtype=np.float32)
    for c, r in enumerate(results):
        b, h = c // 2, c % 2
        o = np.asarray(r['outp'], dtype=np.float32)  # [grp, ll, lt, vv]
        o = o.reshape(8, P, 2, 8 * VC).transpose(0, 3, 2, 1)  # grp, vv, lt, ll
        out[b, :, h * LL:(h + 1) * LL] = o.reshape(NV, LL)
    return out


def run(inputs, trace=False, taps=(), trace_kwargs=None):
    key = ('prog', tuple(sorted(taps)))
    if key not in _CACHE:
        _CACHE[key] = build_program(taps=taps)
    nc = _CACHE[key]
    in_maps = prep_inputs(inputs)
    res = run_bass_kernel_spmd(nc, in_maps, list(range(N_CORES)),
                               trace=trace, **(trace_kwargs or {}))
    return res


def kernel(**inputs):
    res = run(inputs, trace=False)
    return assemble(res.results)
